# revision 1
# baseline (speedup 1.0000x reference)
"""FARNN forward kernel for 8x Trainium2 NeuronCores (Bass/Tile).

Problem (hardcoded):
  B=256, L=512, V=50000, D=300, R=150, SAS=200, fp32 in/out.
  out[b, t, :] = h_t where h_t = relu(W2 @ (L_t * (W1.T @ h_{t-1})) + Ww.T @ h_{t-1})
  L_t = embed_r[tok]*beta + relu(emb[tok] @ (Wg * (1-beta)))     (per (b, t) token)

Sharding: data-parallel over batch. Core c handles batch rows [32c, 32c+32).
FSA weights + embedding tables replicated on every core.

Per-core pipeline (all state-major: feature dims on SBUF partitions):
  - indirect-DMA gather of embedding rows (token-major), cast fp32->fp16
  - DMA(xbar)-transpose to feature-major, fp16 matmul vs folded Wg -> psum
  - fused relu+beta-combine (DVE scalar_tensor_tensor) -> L_all fp16 buffer
  - 512-step recurrence: 12 fp16 matmuls + 1 DVE mult + 1 ACT relu per
    chain-step; NCHAINS independent batch sub-chains hide cross-engine latency
  - h states accumulate in an SBUF staging buffer (also the matmul rhs for the
    next step) and flush to HBM every 16 steps.
Host only shards/reshapes inputs and transposes/concats the outputs.
"""

import numpy as np

import concourse.bass as bass
import concourse.bacc as bacc_mod
import concourse.mybir as mybir
import concourse.tile as tile
from concourse.bass import IndirectOffsetOnAxis

F32 = mybir.dt.float32
F16 = mybir.dt.float16
I32 = mybir.dt.int32

B, L, V, D, R, SAS = 256, 512, 50000, 300, 150, 200
NCORES = 8
BC = B // NCORES          # 32 batch rows per core
GSTEPS = 16               # steps per staging tile / per FF group
NGROUPS = L // GSTEPS     # 32
TOK = BC * L              # tokens per core (16384)
TPG = BC * GSTEPS         # tokens per FF group (512)
NCHUNK = TOK // 128       # 128-token gather chunks (128)
CPG = TPG // 128          # gather chunks per group (4)
DP = 384                  # D padded to xbar multiple (3x128)
RP = 256                  # R padded (2x128)
KD = (128, 128, 44)       # D contraction chunks
KS = (128, 72)            # SAS contraction chunks
KR = (128, 22)            # R contraction chunks


def build_program(nsteps=L, nchains=1, skip_ff=False, skip_rec=False, gbufs=4, tbufs=3, prefetch=3):
    """Emit the full per-core program. Returns nc."""
    nc = bacc_mod.Bacc("TRN2", target_bir_lowering=False, debug=False)
    ngroups = nsteps // GSTEPS
    ch = BC // nchains  # batch per chain (16)

    # ---------------- DRAM I/O ----------------
    idx_d = nc.dram_tensor("idx", [128, NCHUNK], I32, kind="ExternalInput").ap()
    table_d = nc.dram_tensor("table", [V, D + R], F32, kind="ExternalInput").ap()
    wg_d = nc.dram_tensor("wg", [D, R], F32, kind="ExternalInput").ap()
    w1_d = nc.dram_tensor("w1", [SAS, R], F32, kind="ExternalInput").ap()
    w2_d = nc.dram_tensor("w2", [SAS, R], F32, kind="ExternalInput").ap()
    ww_d = nc.dram_tensor("ww", [SAS, SAS], F32, kind="ExternalInput").ap()
    beta_d = nc.dram_tensor("beta", [128, R], F32, kind="ExternalInput").ap()
    outa_d = nc.dram_tensor("outa", [128, nsteps, BC], F16, kind="ExternalOutput").ap()
    outb_d = nc.dram_tensor("outb", [72, nsteps, BC], F16, kind="ExternalOutput").ap()

    from contextlib import ExitStack
    with tile.TileContext(nc) as tc, ExitStack() as ctx:
        consts = ctx.enter_context(tc.tile_pool(name="consts", bufs=1))
        setup = ctx.enter_context(tc.tile_pool(name="setup", bufs=1))

        # ---------------- setup: weights to fp16 SBUF ----------------
        idx_sb = consts.tile([128, NCHUNK], I32)
        nc.sync.dma_start(idx_sb[:], idx_d[:])

        beta_sb = consts.tile([128, R], F32)
        nc.sync.dma_start(beta_sb[:], beta_d[:])
        beta_rep = beta_sb.rearrange("p (c r) -> p c r", c=1).to_broadcast([128, CPG, R])
        omb_sb = consts.tile([128, R], F32)  # 1 - beta
        ones = setup.tile([128, R], F32)
        nc.vector.memset(ones[:], 1.0)
        nc.vector.tensor_sub(omb_sb[:], ones[:], beta_sb[:])

        # W1 [SAS, R] -> fp16 [128, 256] zero-padded cols (M chunks 128+128pad).
        w1_16 = []
        for i, k in enumerate(KS):
            w1_f32 = setup.tile([128, R], F32, name=f"w1f{i}")
            nc.sync.dma_start(w1_f32[:k, :], w1_d[i * 128 : i * 128 + k, :])
            t = consts.tile([128, 256], F16, name=f"w1h{i}")
            nc.vector.memset(t[:], 0.0)
            nc.vector.tensor_copy(t[:k, :R], w1_f32[:k, :])
            w1_16.append(t)

        # Ww [SAS, SAS] -> fp16 [128, 256] zero-padded cols.
        ww_16 = []
        for i, k in enumerate(KS):
            ww_f32 = setup.tile([128, SAS], F32, name=f"wwf{i}")
            nc.sync.dma_start(ww_f32[:k, :], ww_d[i * 128 : i * 128 + k, :])
            t = consts.tile([128, 256], F16, name=f"wwh{i}")
            nc.vector.memset(t[:], 0.0)
            nc.vector.tensor_copy(t[:k, :SAS], ww_f32[:k, :])
            ww_16.append(t)

        # W2T = W2.T as lhsT [K=R-chunk, M=SAS(pad 256)] via DMA transpose.
        w2_16 = []
        for i, k in enumerate(KS):
            w2_f32 = setup.tile([128, R], F32, name=f"w2f{i}")
            nc.sync.dma_start(w2_f32[:k, :], w2_d[i * 128 : i * 128 + k, :])
            t = setup.tile([128, RP], F16, name=f"w2h{i}")
            nc.vector.memset(t[:], 0.0)
            nc.vector.tensor_copy(t[:k, :R], w2_f32[:k, :])
            w2_16.append(t)
        w2t = []
        for j in range(2):  # R chunks
            t = consts.tile([128, 256], F16, name=f"w2t{j}")
            nc.vector.memset(t[:], 0.0)
            w2t.append(t)
        for i in range(2):  # source SAS chunk i -> dest cols (pad rows are zero)
            for j in range(2):  # source col block j -> dest R chunk j
                nc.sync.dma_start_transpose(
                    out=w2t[j][:, i * 128 : (i + 1) * 128],
                    in_=w2_16[i][:, j * 128 : (j + 1) * 128],
                )

        # Wg' = Wg * (1-beta) -> fp16 [128, 256] zero-padded lhsT tiles per K(D)-chunk.
        wg_16 = []
        for i, k in enumerate(KD):
            wg_f32 = setup.tile([128, R], F32, name=f"wgf{i}")
            nc.sync.dma_start(wg_f32[:k, :], wg_d[i * 128 : i * 128 + k, :])
            t = consts.tile([128, 256], F16, name=f"wgh{i}")
            nc.vector.memset(t[:], 0.0)
            nc.vector.tensor_tensor(
                out=t[:k, :R],
                in0=wg_f32[:k, :],
                in1=omb_sb[:k, :],
                op=mybir.AluOpType.mult,
            )
            wg_16.append(t)

        # h0 one-hot block (same layout as a staging step-block).
        h0 = consts.tile([128, 64], F16)
        nc.vector.memset(h0[:], 0.0)
        nc.vector.memset(h0[0:1, 0:32], 1.0)

        # DRAM staging for token-major fp16 [E(384) | betaR(256)] rows.
        dram_pool = ctx.enter_context(tc.tile_pool(name="dstage", bufs=1, space="DRAM"))
        stage_d = dram_pool.tile([TOK, 640], F16)

        # L_all fp16 buffer: per step 64 cols [a(32) | b(32)], a=R 0:128, b=R 128:150.
        lall_pool = ctx.enter_context(tc.tile_pool(name="lall", bufs=1))
        lall = lall_pool.tile([128, 64 * nsteps], F16)
        if skip_ff:
            nc.vector.memset(lall[:], 0.001)
        lall_r = lall.rearrange("p (t c q) -> p t c q", c=2, q=32)

        # ---------------- pools ----------------
        gpool = ctx.enter_context(tc.tile_pool(name="gather", bufs=gbufs))
        tpool = ctx.enter_context(tc.tile_pool(name="trans", bufs=tbufs))
        ff_psum = ctx.enter_context(tc.tile_pool(name="ffpsum", bufs=1, space="PSUM"))
        rec_psum = ctx.enter_context(tc.tile_pool(name="recpsum", bufs=1, space="PSUM"))
        hw_psum = ctx.enter_context(tc.tile_pool(name="hwpsum", bufs=2, space="PSUM"))
        xpool = ctx.enter_context(tc.tile_pool(name="xpool", bufs=3))
        stage_pool = ctx.enter_context(tc.tile_pool(name="stage", bufs=3))

        stage_tiles = {}

        def ff_gather_group(g):
            """Gather+cast group g (CPG chunks) into DRAM staging."""
            g32 = gpool.tile([128, CPG, D + R], F32, name="g32", tag="g32")
            for s in range(CPG):
                nc.gpsimd.indirect_dma_start(
                    out=g32[:, s, :], out_offset=None, in_=table_d[:],
                    in_offset=IndirectOffsetOnAxis(ap=idx_sb[:, g * CPG + s : g * CPG + s + 1], axis=0),
                )
            er16 = gpool.tile([128, CPG, 640], F16, name="er16", tag="er16")
            nc.scalar.copy(er16[:, :, :D], g32[:, :, :D])
            nc.vector.memset(er16[:, :, D:DP], 0.0)
            nc.vector.tensor_tensor(
                out=er16[:, :, DP : DP + R], in0=g32[:, :, D : D + R],
                in1=beta_rep[:, :, :],
                op=mybir.AluOpType.mult,
            )
            nc.vector.memset(er16[:, :, DP + R : 640], 0.0)
            nc.scalar.dma_start(
                stage_d[g * TPG : (g + 1) * TPG, :].rearrange("(c p) f -> p c f", p=128),
                er16[:],
            )

        def ff_group(g):
            """Produce L_all columns for steps [g*GSTEPS, (g+1)*GSTEPS)."""
            et = [tpool.tile([128, TPG], F16, name=f"et{k}", tag=f"et{k}") for k in range(3)]
            rt = [tpool.tile([128, TPG], F16, name=f"rt{k}", tag=f"rt{k}") for k in range(2)]
            rows = stage_d[g * TPG : (g + 1) * TPG, :]
            for k in range(3):
                eng = nc.sync if k % 2 == 0 else nc.scalar
                eng.dma_start_transpose(out=et[k][:], in_=rows[:, k * 128 : (k + 1) * 128])
            for k in range(2):
                eng = nc.scalar if k % 2 == 0 else nc.sync
                eng.dma_start_transpose(out=rt[k][:], in_=rows[:, DP + k * 128 : DP + (k + 1) * 128])
            # FF matmul: psum_a [128, TPG] = relu-pending Wg'.T @ embT (R 0:128)
            pa = ff_psum.tile([128, TPG], F32, name="ffpa", tag="ffpa", space="PSUM")
            pb = ff_psum.tile([128, TPG], F32, name="ffpb", tag="ffpb", space="PSUM")
            for k in range(3):
                nc.tensor.matmul(
                    pa[:], wg_16[k][: KD[k], 0:128], et[k][: KD[k], :],
                    start=(k == 0), stop=(k == 2),
                )
            for k in range(3):
                nc.tensor.matmul(
                    pb[:], wg_16[k][: KD[k], 128:256], et[k][: KD[k], :],
                    start=(k == 0), stop=(k == 2),
                )
            # combine: L_all = relu(psum) + beta*embr   (stt: max(in0,0) add in1)
            t0 = g * GSTEPS
            nc.vector.scalar_tensor_tensor(
                out=lall_r[:, t0 : t0 + GSTEPS, 0, :],
                in0=pa[:].rearrange("p (t q) -> p t q", q=BC),
                scalar=0.0,
                in1=rt[0][:, :].rearrange("p (t q) -> p t q", q=BC),
                op0=mybir.AluOpType.max,
                op1=mybir.AluOpType.add,
            )
            nc.vector.scalar_tensor_tensor(
                out=lall_r[:, t0 : t0 + GSTEPS, 1, :],
                in0=pb[:].rearrange("p (t q) -> p t q", q=BC),
                scalar=0.0,
                in1=rt[1][:, :].rearrange("p (t q) -> p t q", q=BC),
                op0=mybir.AluOpType.max,
                op1=mybir.AluOpType.add,
            )

        def rec_step(t, q):
            """One recurrence step for chain q."""
            if t == 0:
                prev = h0
                j = 0
            else:
                prev = stage_tiles[(t - 1) // GSTEPS]
                j = (t - 1) % GSTEPS
            qa = q * ch          # offset within a 32-col a/b block
            pk1 = prev[0:128, 64 * j + qa : 64 * j + qa + ch]
            pk2 = prev[0:72, 64 * j + 32 + qa : 64 * j + 32 + qa + ch]

            # Rh = W1.T @ h   (a: R 0:128, b: R 128:256pad) - separate psum groups
            prh_a = rec_psum.tile([128, ch], F32, name="prh_a", tag="prh_a", space="PSUM")
            prh_b = rec_psum.tile([128, ch], F32, name="prh_b", tag="prh_b", space="PSUM")
            nc.tensor.matmul(prh_a[:], w1_16[0][:, 0:128], pk1, start=True, stop=False)
            nc.tensor.matmul(prh_a[:], w1_16[1][:72, 0:128], pk2, start=False, stop=True)
            nc.tensor.matmul(prh_b[:], w1_16[0][:, 128:256], pk1, start=True, stop=False)
            nc.tensor.matmul(prh_b[:], w1_16[1][:72, 128:256], pk2, start=False, stop=True)

            # wild = Ww.T @ h (lang accumulates later)
            phw_a = hw_psum.tile([128, ch], F32, name="phw_a", tag="phw_a", space="PSUM")
            phw_b = hw_psum.tile([128, ch], F32, name="phw_b", tag="phw_b", space="PSUM")
            nc.tensor.matmul(phw_a[:], ww_16[0][:, 0:128], pk1, start=True, stop=False)
            nc.tensor.matmul(phw_a[:], ww_16[1][:72, 0:128], pk2, start=False, stop=False)
            nc.tensor.matmul(phw_b[:], ww_16[0][:, 128:256], pk1, start=True, stop=False)
            nc.tensor.matmul(phw_b[:], ww_16[1][:72, 128:256], pk2, start=False, stop=False)

            # X = L_t * Rh  (fp16), split a/b so lang-K1 can start early
            x16 = xpool.tile([128, 2 * ch], F16, name="x16", tag="x16")
            nc.vector.tensor_tensor(
                out=x16[:, 0:ch], in0=prh_a[:],
                in1=lall_r[:, t, 0, qa : qa + ch],
                op=mybir.AluOpType.mult,
            )
            nc.vector.tensor_tensor(
                out=x16[:, ch : 2 * ch], in0=prh_b[:],
                in1=lall_r[:, t, 1, qa : qa + ch],
                op=mybir.AluOpType.mult,
            )

            # lang = W2T.T @ X accumulated into phw
            nc.tensor.matmul(phw_a[:], w2t[0][:, 0:128], x16[0:128, 0:ch], start=False, stop=False)
            nc.tensor.matmul(phw_a[:], w2t[1][:22, 0:128], x16[0:22, ch : 2 * ch], start=False, stop=True)
            nc.tensor.matmul(phw_b[:], w2t[0][:, 128:256], x16[0:128, 0:ch], start=False, stop=False)
            nc.tensor.matmul(phw_b[:], w2t[1][:22, 128:256], x16[0:22, ch : 2 * ch], start=False, stop=True)

            # h = relu(phw) -> staging (fp16): a on DVE (fast, feeds K1 mms), b on ACT
            cur = stage_tiles[t // GSTEPS]
            cur_r = cur.rearrange("p (t c q) -> p t c q", c=2, q=32)
            nc.vector.tensor_scalar_max(
                cur_r[:, t % GSTEPS, 0, qa : qa + ch], phw_a[:], 0.0,
            )
            nc.scalar.activation(
                out=cur_r[:, t % GSTEPS, 1, qa : qa + ch],
                in_=phw_b[:],
                func=mybir.ActivationFunctionType.Relu,
            )

        def flush_group(g):
            st = stage_tiles[g]
            st_r = st.rearrange("p (t c q) -> p t c q", c=2, q=32)
            t0 = g * GSTEPS
            nc.scalar.dma_start(outa_d[:, t0 : t0 + GSTEPS, :], st_r[:, :, 0, :])
            nc.scalar.dma_start(outb_d[:, t0 : t0 + GSTEPS, :], st_r[0:72, :, 1, :])

        PREFETCH = prefetch
        done_g = 0

        def ff_upto(gmax):
            nonlocal done_g
            while done_g < min(gmax, ngroups):
                ff_gather_group(done_g)
                done_g += 1

        for g in range(min(PREFETCH, ngroups)):
            if not skip_ff:
                ff_upto(g + 1)
                ff_group(g)
        for g in range(ngroups):
            if g + PREFETCH < ngroups and not skip_ff:
                ff_upto(g + PREFETCH + 2)
                ff_group(g + PREFETCH)
            stage_tiles[g] = stage_pool.tile([128, 64 * GSTEPS], F16, name="stage", tag="stage")
            if not skip_rec:
                for t in range(g * GSTEPS, (g + 1) * GSTEPS):
                    for q in range(nchains):
                        rec_step(t, q)
            else:
                nc.vector.memset(stage_tiles[g][:], 0.0)
            flush_group(g)

    nc.compile()
    return nc


def _prep_core_inputs(core, input_i32, table, wg, w1, w2, ww, beta):
    bsl = slice(core * BC, (core + 1) * BC)
    shard = input_i32[bsl]                       # [BC, L]
    idx_tm = np.ascontiguousarray(shard.T).reshape(-1)   # t-major tokens [L*BC]
    idx_pc = np.ascontiguousarray(idx_tm.reshape(NCHUNK, 128).T)  # [128, NCHUNK]
    return {
        "idx": idx_pc,
        "table": table, "wg": wg,
        "w1": w1, "w2": w2, "ww": ww,
        "beta": np.ascontiguousarray(np.broadcast_to(beta.reshape(1, R), (128, R))),
    }


def kernel(input, lengths, embedding, embed_r, embed_r_generalized,
           trans_r_1, trans_r_2, trans_wildcard, beta_vec, _nc_cache={}):
    input_i32 = np.ascontiguousarray(np.asarray(input).astype(np.int32))
    emb = np.ascontiguousarray(np.asarray(embedding, dtype=np.float32))
    embr = np.ascontiguousarray(np.asarray(embed_r, dtype=np.float32))
    wg = np.ascontiguousarray(np.asarray(embed_r_generalized, dtype=np.float32))
    w1 = np.ascontiguousarray(np.asarray(trans_r_1, dtype=np.float32))
    w2 = np.ascontiguousarray(np.asarray(trans_r_2, dtype=np.float32))
    ww = np.ascontiguousarray(np.asarray(trans_wildcard, dtype=np.float32))
    beta = np.ascontiguousarray(np.asarray(beta_vec, dtype=np.float32))

    if "nc" not in _nc_cache:
        _nc_cache["nc"] = build_program()
    nc = _nc_cache["nc"]

    table = np.ascontiguousarray(np.concatenate([emb, embr], axis=1))
    in_maps = [
        _prep_core_inputs(c, input_i32, table, wg, w1, w2, ww, beta)
        for c in range(NCORES)
    ]

    from concourse import bass_utils
    res = bass_utils.run_bass_kernel_spmd(nc, in_maps, core_ids=list(range(NCORES)))

    out = np.empty((B, L, SAS), np.float32)
    for c in range(NCORES):
        full = np.concatenate(
            [res.results[c]["outa"], res.results[c]["outb"]], axis=0
        )  # [200, L, BC] fp16
        out[c * BC : (c + 1) * BC] = full.transpose(2, 1, 0).astype(np.float32)
    return out


if __name__ == "__main__":
    import reference

    inputs = {k: np.asarray(v) for k, v in reference.setup_inputs().items()}
    got = kernel(**inputs)
    print("kernel output:", got.shape, got.dtype)

